# revision 30
# baseline (speedup 1.0000x reference)
"""MoE layer (E=8 experts, top-2 routing) on 8 Trainium2 NeuronCores.

Expert-parallel sharding: core e holds expert e's weights (w1/w2/b1).
Tokens are dispatched (host-side router, exact fp32) to the cores of their
top-2 experts; each core runs its expert's FFN on its gathered tokens in
bf16 (h = relu(x@w1+b1); out = h@w2). The bias b2 and the combine weights
are folded into the host-side scatter-add return.

Shapes (hardcoded per the problem spec):
  x [2, 2048, 512] f32, router_w [8, 512], w1_all [8, 2048, 512],
  b1_all [8, 2048], w2_all [8, 512, 2048], b2_all [8, 512].

Device program per core (bf16 operands, fp32 PSUM), cap ~1096 tokens
(max expert count; a load-balanced variant with a secondary expert block
was measured net-negative: its +4MB weight DMA ate the 2.4us PE saving):
  stage 1/2: mm1 on block0 (512 tok), k-major over m-quads so the first
    matmul needs only ~250KB of DMA (xg k-slice + one w1 chunk) instead of
    a full 1.25MB m-major prefetch -- PE starts ~2.5us into the window.
  stage 3:   mm1 on block1 (512) + tail (cap-1024) interleaved per m;
    w1 is fully resident by then so this phase has zero input-DMA demand,
    which is when w2 streams in.
  stage 4:   mm2 per 128-token tile (h stationary, w2 moving) + transposed
    mm2 for the sub-128 remainder; last full tile is d-split so its
    cast+DMA overlaps the final matmuls.
All relu (+bias) and casts run on DVE, so the scalar engine never loads an
ACT table and is a pure DMA issuer from t=0 (3 issue queues: sync/gpsimd/
scalar). Warmup matmuls on a memset tile ramp the HAM clock (1.2->2.4GHz)
during the DMA head. PE floor = cap*128 cycles = 58.5us at cap=1096.
"""

import sys

sys.path.insert(0, "/opt/trn_rl_repo")

import numpy as np
import ml_dtypes

import concourse.mybir as mybir
import concourse.tile as tile
from concourse import bacc

D_MODEL = 512
DFF = 2048
E = 8
K = 2
L = 2 * 2048  # total tokens
N_CORES = 8

FP = mybir.dt.float32
BF = mybir.dt.bfloat16
NPBF = ml_dtypes.bfloat16

# Per-expert token capacity (padded). Expected load is L*K/E = 1024 with
# std ~30 under the near-uniform router; seed-0 max count is 1094. The
# program is built for the actual max count (rounded to a multiple of 4).
CAP = 1092

KD = D_MODEL // 128  # 4 k-slices (contraction for mm1)
MD = DFF // 128  # 16 dff tiles

_PROG_CACHE: dict = {}


def build_program(cap: int, warm: int = 12):
    """One SPMD program, run on all 8 cores; per-core data selects the expert.

    Per-core inputs (bf16 except b1d):
      xgd  [512, cap]    gathered tokens, d-major (row k*128+p = d, col t)
      w1d  [128, 8192]   w1 k-major merged: [p, k*2048 + m*128 + c]
                         = w1_e[m*128+c, k*128+p]
      w2d  [128, 8192]   w2.T m-major merged: [p, m*512 + d]
                         = w2_e[d, m*128+p]
      b1d  [128, 16]     b1_e: [p, m] = b1_e[m*128+p]  (fp32)
    Outputs:
      out  [(cap//128)*128, 512] bf16 expert FFN outputs, row s = slot s
      outt [128, 4, rem] bf16 transposed remainder (d = dj*128 + p)
    """
    nc = bacc.Bacc("TRN2", target_bir_lowering=False, debug=False)

    xgd = nc.dram_tensor("xgd", [D_MODEL, cap], BF, kind="ExternalInput")
    w1d = nc.dram_tensor("w1d", [128, KD * DFF], BF, kind="ExternalInput")
    w2d = nc.dram_tensor("w2d", [128, MD * D_MODEL], BF, kind="ExternalInput")
    b1d = nc.dram_tensor("b1d", [128, MD], FP, kind="ExternalInput")
    full_cols = (cap // 128) * 128
    rem = cap - full_cols  # sub-128 remainder (transposed mm2 path)
    assert 0 < rem < 128
    out = nc.dram_tensor("out", [full_cols, D_MODEL], BF, kind="ExternalOutput")
    outt = nc.dram_tensor("outt", [128, KD, rem], BF, kind="ExternalOutput")

    B0 = 512
    B1 = 512
    BT = cap - B0 - B1  # tail block (includes the sub-128 remainder)
    n_full_tiles = full_cols // 128

    with tile.TileContext(nc) as tc:
        with (
            tc.tile_pool(name="weights", bufs=1) as wpool,
            tc.tile_pool(name="h", bufs=1) as hpool,
            tc.tile_pool(name="psum", bufs=1, space="PSUM") as ppool,
            tc.tile_pool(name="outp", bufs=3) as opool,
            tc.tile_pool(name="consts", bufs=1) as cpool,
        ):
            xg_sb = wpool.tile([128, KD, cap], BF, tag="xg")
            w1_sb = wpool.tile([128, KD * DFF], BF, tag="w1")
            w2_sb = wpool.tile([128, MD * D_MODEL], BF, tag="w2")
            b1_sb = wpool.tile([128, MD], FP, tag="b1")
            warm_sb = cpool.tile([128, 512], BF, tag="warm")
            h_sb = [
                hpool.tile([128, cap], BF, tag=f"h{m}", name=f"h_{m}")
                for m in range(MD)
            ]

            # memset on DVE (idle at entry): gpsimd's first DMA issue isn't
            # delayed, and the warm matmuls (which gate the HAM clock ramp)
            # start ~0.5us earlier.
            nc.vector.memset(warm_sb[:], 0.0)

            # --- DMA issue schedule -------------------------------------
            # 3 issue queues (sync + scalar are HW DGE, gpsimd SW DGE).
            # Issue cost is ~0.6us of engine time each; transfers serialize
            # per queue in issue order, so each queue's stream is kept in
            # need order and the two HW queues carry the startup-critical
            # chunks (gpsimd's head is delayed by the warm memset).
            def xg_src(k, lo, hi):
                return xgd[k * 128 : (k + 1) * 128, lo:hi]

            def w1_ap(k, mlo, mhi):
                return (
                    w1_sb[:, k * DFF + mlo * 128 : k * DFF + mhi * 128],
                    w1d[:, k * DFF + mlo * 128 : k * DFF + mhi * 128],
                )

            def w1_dma(eng, k, mlo, mhi):
                dst, src = w1_ap(k, mlo, mhi)
                eng.dma_start(out=dst, in_=src)

            def xg_dma(eng, k, lo, hi):
                eng.dma_start(out=xg_sb[:, k, lo:hi], in_=xg_src(k, lo, hi))

            def w2_dma(eng, lo, hi):
                eng.dma_start(out=w2_sb[:, lo:hi], in_=w2d[:, lo:hi])

            S, A, G = nc.sync, nc.scalar, nc.gpsimd
            # round 1-3: stage-1 quad0 critical set + quad1 weights
            w1_dma(S, 0, 0, 4); xg_dma(A, 0, 0, B0); xg_dma(G, 1, 0, B0)
            xg_dma(S, 2, 0, B0); w1_dma(A, 1, 0, 4); xg_dma(G, 3, 0, B0)
            w1_dma(S, 2, 0, 4); w1_dma(A, 3, 0, 4)
            G.dma_start(out=b1_sb[:], in_=b1d[:, :])
            # round 4: quad1 (m4-7)
            w1_dma(S, 0, 4, 8); w1_dma(A, 1, 4, 8); w1_dma(G, 2, 4, 8)
            # round 5-6: stage-2 weights (m8-15, 256KB per k)
            w1_dma(S, 3, 4, 8); w1_dma(A, 0, 8, 16); w1_dma(G, 1, 8, 16)
            w1_dma(S, 2, 8, 16); w1_dma(A, 3, 8, 16)
            # round 6-7: stage-3a tokens (needed ~25us in)
            xg_dma(G, 0, B0, B0 + B1)
            xg_dma(S, 1, B0, B0 + B1); xg_dma(A, 2, B0, B0 + B1)
            xg_dma(G, 3, B0, B0 + B1)
            xg_dma(S, 0, B0 + B1, cap); xg_dma(A, 1, B0 + B1, cap)
            xg_dma(G, 2, B0 + B1, cap); xg_dma(S, 3, B0 + B1, cap)
            # stage-4 weights (needed ~40us in)
            w2_dma(A, 0, 2048); w2_dma(S, 2048, 4096)
            w2_dma(G, 4096, 6144); w2_dma(A, 6144, 8192)

            # --- PE warmup: ramp the HAM clock during the DMA head ------
            ps_w = ppool.tile([128, 512], FP, tag="W", bufs=1, name="ps_warm")
            for i in range(warm):
                # alternate halves of the bank so back-to-back warms have
                # no WAW dependency -- gaps would pause the HAM ramp timer
                half = (i % 2) * 256
                nc.tensor.matmul(
                    ps_w[:, half : half + 256],
                    warm_sb[:, :128],
                    warm_sb[:, :256],
                    start=True,
                    stop=True,
                )
            if warm:
                warm_sink = cpool.tile([1, 8], FP, tag="warm_sink")
                nc.vector.tensor_copy(warm_sink[:], ps_w[0:1, 0:8])

            def relu(m, ps, lo, hi, bias=None):
                """h[m][:, lo:hi] = max(ps + bias[m], 0) in bf16 on DVE.

                DVE-only: scalar must stay a pure DMA issuer -- Tile keeps
                per-engine emission order, so relus on scalar would queue
                behind all of its DMA issues (measured: first ACT relu at
                27us, PSUM backpressure, clock down-throttle, +9us)."""
                nc.vector.tensor_scalar(
                    h_sb[m][:, lo:hi],
                    ps[:, : hi - lo],
                    (bias if bias is not None else b1_sb)[:, m : m + 1],
                    0.0,
                    mybir.AluOpType.add,
                    mybir.AluOpType.max,
                )

            # --- stage 1+2: mm1 on block0, k-major over m-quads ---------
            # Per k-pass a quad consumes one 128KB w1 chunk (+128KB xg on
            # the first quad) -- matched to the cold DMA/clock ramp.
            for q in range(4):
                ps_q = [
                    ppool.tile([128, B0], FP, tag="A", bufs=7, name=f"ps_{q}_{i}")
                    for i in range(4)
                ]
                for k in range(KD):
                    for i in range(4):
                        m = q * 4 + i
                        nc.tensor.matmul(
                            ps_q[i][:],
                            w1_sb[:, k * DFF + m * 128 : k * DFF + (m + 1) * 128],
                            xg_sb[:, k, 0:B0],
                            start=(k == 0),
                            stop=(k == KD - 1),
                        )
                    if warm and q == 0:
                        # thin deterministic padding (128-col, ~53ns): keeps
                        # the clock ramp alive if quad0 stalls on DMA, at
                        # negligible cost when it doesn't
                        nc.tensor.matmul(
                            ps_w[:, :128], warm_sb[:, :128], warm_sb[:, :128],
                            start=True, stop=True,
                        )
                for i in range(4):
                    relu(q * 4 + i, ps_q[i], 0, B0)

            # --- stage 3: mm1 on block1 + tail, m-major (w1 resident) ---
            for m in range(MD):
                ps_b = ppool.tile(
                    [128, B1], FP, tag="A", bufs=7, name=f"ps3_{m}"
                )
                for k in range(KD):
                    nc.tensor.matmul(
                        ps_b[:],
                        w1_sb[:, k * DFF + m * 128 : k * DFF + (m + 1) * 128],
                        xg_sb[:, k, B0 : B0 + B1],
                        start=(k == 0),
                        stop=(k == KD - 1),
                    )
                ps_t = ppool.tile(
                    [128, BT], FP, tag="A", bufs=7, name=f"ps3t_{m}"
                )
                for k in range(KD):
                    nc.tensor.matmul(
                        ps_t[:],
                        w1_sb[:, k * DFF + m * 128 : k * DFF + (m + 1) * 128],
                        xg_sb[:, k, B0 + B1 : cap],
                        start=(k == 0),
                        stop=(k == KD - 1),
                    )
                relu(m, ps_b, B0, B0 + B1)
                relu(m, ps_t, B0 + B1, cap)

            # --- stage 4: mm2 ------------------------------------------
            def emit_mm2(t, chunks=((0, 512),), engs=(nc.scalar,)):
                o = opool.tile([128, D_MODEL], BF, tag="o")
                for ci, (dlo, dhi) in enumerate(chunks):
                    ps2 = ppool.tile(
                        [128, dhi - dlo], FP, tag="A", bufs=7, name=f"ps2_{t}"
                    )
                    for m in range(MD):
                        nc.tensor.matmul(
                            ps2[:],
                            h_sb[m][:, t * 128 : (t + 1) * 128],
                            w2_sb[:, m * 512 + dlo : m * 512 + dhi],
                            start=(m == 0),
                            stop=(m == MD - 1),
                        )
                    nc.vector.tensor_copy(o[:, dlo:dhi], ps2[:])
                    engs[ci % len(engs)].dma_start(
                        out=out[t * 128 : (t + 1) * 128, dlo:dhi],
                        in_=o[:, dlo:dhi],
                    )

            for t in range(n_full_tiles - 1):
                emit_mm2(t)
            # transposed remainder next-to-last: its DMA (slow gpsimd
            # queue) issues while the last tile's matmuls still run
            oT = opool.tile([128, KD, rem], BF, tag="oT", bufs=1)
            for dj in range(KD):
                psT = ppool.tile(
                    [128, rem], FP, tag="A", bufs=7, name=f"psT_{dj}"
                )
                for m in range(MD):
                    nc.tensor.matmul(
                        psT[:],
                        w2_sb[:, m * 512 + dj * 128 : m * 512 + (dj + 1) * 128],
                        h_sb[m][:, full_cols:cap],
                        start=(m == 0),
                        stop=(m == MD - 1),
                    )
                nc.vector.tensor_copy(oT[:, dj, :], psT[:])
            nc.gpsimd.dma_start(out=outt[:, :, :], in_=oT[:])
            # last full tile in 4 d-chunks so each chunk's cast+DMA
            # overlaps the next chunk's matmuls; final transfers are small
            # and ride the fast HW queues
            emit_mm2(
                n_full_tiles - 1,
                chunks=((0, 192), (192, 320), (320, 448), (448, 512)),
                engs=(nc.scalar, nc.sync, nc.scalar, nc.sync),
            )
    nc.compile()
    return nc


def _route(x_flat: np.ndarray, router_w: np.ndarray):
    """Host-side replica of the reference router: top-2 + renormalized weights."""
    logits = x_flat @ router_w.T  # [L, E]
    m = logits.max(axis=-1, keepdims=True)
    p = np.exp(logits - m)
    p /= p.sum(axis=-1, keepdims=True)
    order = np.argsort(-p, axis=-1)[:, :K]  # [L, K]
    pv = np.take_along_axis(p, order, axis=-1)
    pv = pv / (pv.sum(axis=-1, keepdims=True) + 1e-9)
    return order, pv


def _stage_weights(w1_all, b1_all, w2_all, e):
    """k-major w1, m-major w2, [p, m] b1 for expert e (see build_program)."""
    w1e = np.asarray(w1_all, np.float32)[e]  # [dff, d]
    w1m = (
        w1e.reshape(MD, 128, KD, 128)  # [m, c, k, p]
        .transpose(3, 2, 0, 1)  # [p, k, m, c]
        .reshape(128, KD * DFF)
    )
    w2e = np.asarray(w2_all, np.float32)[e]  # [d, dff]
    w2m = (
        w2e.reshape(D_MODEL, MD, 128)  # [d, m, p]
        .transpose(2, 1, 0)  # [p, m, d]
        .reshape(128, MD * D_MODEL)
    )
    b1e = np.asarray(b1_all, np.float32)[e].reshape(MD, 128).T  # [p, m]
    return (
        np.ascontiguousarray(w1m).astype(NPBF),
        np.ascontiguousarray(w2m).astype(NPBF),
        np.ascontiguousarray(b1e),
    )


def _build_in_maps(x, router_w, w1_all, b1_all, w2_all, b2_all):
    """Shared staging: router + expert-parallel dispatch + per-core maps.

    Returns (cap, in_maps, idx_lists, wgt) where idx_lists[e] = token idx
    array for core e and wgt[t, e] = combine weight of expert e for token t.
    """
    x_flat = np.asarray(x, np.float32).reshape(-1, D_MODEL)
    order, pv = _route(x_flat, np.asarray(router_w, np.float32))
    idx_lists, wgt = [], np.zeros((x_flat.shape[0], E), np.float32)
    for e in range(E):
        sel = np.nonzero(order == e)
        idx_lists.append(sel[0])
        wgt[sel[0], e] = pv[sel]
    max_n = max(len(t) for t in idx_lists)
    cap = max(CAP, -(-max_n // 4) * 4)
    in_maps = []
    for e in range(E):
        toks = idx_lists[e]
        xg = np.zeros((cap, D_MODEL), np.float32)
        xg[: len(toks)] = x_flat[toks]
        w1m, w2m, b1e = _stage_weights(w1_all, b1_all, w2_all, e)
        in_maps.append(
            {
                "xgd": np.ascontiguousarray(xg.T).astype(NPBF),
                "w1d": w1m,
                "w2d": w2m,
                "b1d": b1e,
            }
        )
    return cap, in_maps, idx_lists, wgt


def _get_program(cap: int):
    if cap not in _PROG_CACHE:
        _PROG_CACHE[cap] = build_program(cap)
    return _PROG_CACHE[cap]


def kernel(x, router_w, w1_all, b1_all, w2_all, b2_all):
    from concourse.bass_utils import run_bass_kernel_spmd

    x = np.asarray(x, dtype=np.float32)
    Bb, Nn, C = x.shape

    cap, in_maps, idx_lists, wgt = _build_in_maps(
        x, router_w, w1_all, b1_all, w2_all, b2_all
    )
    nc = _get_program(cap)

    for _attempt in range(3):
        res = run_bass_kernel_spmd(nc, in_maps, core_ids=list(range(N_CORES)))
        outs = []
        bad = False
        for e in range(E):
            o = res.results[e]["out"].astype(np.float32)
            ot = res.results[e]["outt"].astype(np.float32)
            # [128, 4, rem] -> [rem, 512] with d = dj*128 + p
            o_tail = np.transpose(ot, (2, 1, 0)).reshape(ot.shape[2], -1)
            o = np.concatenate([o, o_tail], axis=0)[: len(idx_lists[e])]
            if np.isnan(o).any():
                bad = True
                break
            outs.append(o)
        if not bad:
            break
    else:
        raise RuntimeError("device output contained NaN after 3 attempts")

    b2f = np.asarray(b2_all, np.float32)
    final = np.zeros((Bb * Nn, C), np.float32)
    for e in range(E):
        toks = idx_lists[e]
        final[toks] += (outs[e] + b2f[e]) * wgt[toks, e][:, None]
    return final.reshape(Bb, Nn, C)


# revision 35
# speedup vs baseline: 1.1826x; 1.1826x over previous
"""MoE layer (E=8 experts, top-2 routing) on 8 Trainium2 NeuronCores.

Expert-parallel sharding: core e holds expert e's weights (w1/w2/b1).
Tokens are dispatched (host-side router, exact fp32) to the cores of their
top-2 experts; each core runs its expert's FFN on its gathered tokens in
bf16 (h = relu(x@w1+b1); out = h@w2). The bias b2 and the combine weights
are folded into the host-side scatter-add return.

Shapes (hardcoded per the problem spec):
  x [2, 2048, 512] f32, router_w [8, 512], w1_all [8, 2048, 512],
  b1_all [8, 2048], w2_all [8, 512, 2048], b2_all [8, 512].

Device program per core (bf16 operands, fp32 PSUM), cap ~1096 tokens
(max expert count; a load-balanced variant with a secondary expert block
was measured net-negative: its +4MB weight DMA ate the 2.4us PE saving):
  stage 1/2: mm1 on block0 (512 tok), k-major over m-quads so the first
    matmul needs only ~250KB of DMA (xg k-slice + one w1 chunk) instead of
    a full 1.25MB m-major prefetch -- PE starts ~2.5us into the window.
  stage 3:   mm1 on block1 (512) + tail (cap-1024) interleaved per m;
    w1 is fully resident by then so this phase has zero input-DMA demand,
    which is when w2 streams in.
  stage 4:   mm2 per 128-token tile (h stationary, w2 moving) + transposed
    mm2 for the sub-128 remainder; last full tile is d-split so its
    cast+DMA overlaps the final matmuls.
All relu (+bias) and casts run on DVE, so the scalar engine never loads an
ACT table and is a pure DMA issuer from t=0 (3 issue queues: sync/gpsimd/
scalar). Warmup matmuls on a memset tile ramp the HAM clock (1.2->2.4GHz)
during the DMA head. PE floor = cap*128 cycles = 58.5us at cap=1096.
"""

import sys

sys.path.insert(0, "/opt/trn_rl_repo")

import numpy as np
import ml_dtypes

import concourse.mybir as mybir
import concourse.tile as tile
from concourse import bacc

D_MODEL = 512
DFF = 2048
E = 8
K = 2
L = 2 * 2048  # total tokens
N_CORES = 8

FP = mybir.dt.float32
BF = mybir.dt.bfloat16
NPBF = ml_dtypes.bfloat16

# Per-expert token capacity (padded). Expected load is L*K/E = 1024 with
# std ~30 under the near-uniform router; seed-0 max count is 1094. The
# program is built for the actual max count (rounded to a multiple of 4).
CAP = 1092

KD = D_MODEL // 128  # 4 k-slices (contraction for mm1)
MD = DFF // 128  # 16 dff tiles

_PROG_CACHE: dict = {}


def build_program(cap: int, warm: int = 9):
    """One SPMD program, run on all 8 cores; per-core data selects the expert.

    Per-core inputs (bf16 except b1d):
      xgd  [512, cap]    gathered tokens, d-major (row k*128+p = d, col t)
      w1d  [128, 8192]   w1 k-major merged: [p, k*2048 + m*128 + c]
                         = w1_e[m*128+c, k*128+p]
      w2d  [128, 8192]   w2.T m-major merged: [p, m*512 + d]
                         = w2_e[d, m*128+p]
      b1d  [128, 16]     b1_e: [p, m] = b1_e[m*128+p]  (fp32)
    Outputs:
      out  [(cap//128)*128, 512] bf16 expert FFN outputs, row s = slot s
      outt [128, 4, rem] bf16 transposed remainder (d = dj*128 + p)
    """
    nc = bacc.Bacc("TRN2", target_bir_lowering=False, debug=False)

    xgd = nc.dram_tensor("xgd", [D_MODEL, cap], BF, kind="ExternalInput")
    w1d = nc.dram_tensor("w1d", [128, KD * DFF], BF, kind="ExternalInput")
    w2d = nc.dram_tensor("w2d", [128, MD * D_MODEL], BF, kind="ExternalInput")
    b1d = nc.dram_tensor("b1d", [128, MD], FP, kind="ExternalInput")
    full_cols = (cap // 128) * 128
    rem = cap - full_cols  # sub-128 remainder (transposed mm2 path)
    assert 0 < rem < 128
    out = nc.dram_tensor("out", [full_cols, D_MODEL], BF, kind="ExternalOutput")
    outt = nc.dram_tensor("outt", [128, KD, rem], BF, kind="ExternalOutput")

    B0 = 512
    B1 = 512
    BT = cap - B0 - B1  # tail block (includes the sub-128 remainder)
    n_full_tiles = full_cols // 128

    with tile.TileContext(nc) as tc:
        with (
            tc.tile_pool(name="weights", bufs=1) as wpool,
            tc.tile_pool(name="h", bufs=1) as hpool,
            tc.tile_pool(name="psum", bufs=1, space="PSUM") as ppool,
            tc.tile_pool(name="outp", bufs=3) as opool,
            tc.tile_pool(name="consts", bufs=1) as cpool,
        ):
            xg_sb = wpool.tile([128, KD, cap], BF, tag="xg")
            w1_sb = wpool.tile([128, KD * DFF], BF, tag="w1")
            w2_sb = wpool.tile([128, MD * D_MODEL], BF, tag="w2")
            b1_sb = wpool.tile([128, MD], FP, tag="b1")
            warm_sb = cpool.tile([128, 256], BF, tag="warm")
            h_sb = [
                hpool.tile([128, cap], BF, tag=f"h{m}", name=f"h_{m}")
                for m in range(MD)
            ]

            # memset on DVE (idle at entry): gpsimd's first DMA issue isn't
            # delayed, and the warm matmuls (which gate the HAM clock ramp)
            # start ~0.5us earlier.
            nc.vector.memset(warm_sb[:], 0.0)

            # --- DMA issue schedule -------------------------------------
            # 3 issue queues (sync + scalar are HW DGE, gpsimd SW DGE).
            # Issue cost is ~0.6us of engine time each; transfers serialize
            # per queue in issue order, so each queue's stream is kept in
            # need order and the two HW queues carry the startup-critical
            # chunks (gpsimd's head is delayed by the warm memset).
            def xg_src(k, lo, hi):
                return xgd[k * 128 : (k + 1) * 128, lo:hi]

            def w1_ap(k, mlo, mhi):
                return (
                    w1_sb[:, k * DFF + mlo * 128 : k * DFF + mhi * 128],
                    w1d[:, k * DFF + mlo * 128 : k * DFF + mhi * 128],
                )

            def w1_dma(eng, k, mlo, mhi):
                dst, src = w1_ap(k, mlo, mhi)
                eng.dma_start(out=dst, in_=src)

            def xg_dma(eng, k, lo, hi):
                eng.dma_start(out=xg_sb[:, k, lo:hi], in_=xg_src(k, lo, hi))

            def w2_dma(eng, lo, hi):
                eng.dma_start(out=w2_sb[:, lo:hi], in_=w2d[:, lo:hi])

            S, A, G = nc.sync, nc.scalar, nc.gpsimd
            # round 1-3: stage-1 quad0 critical set + quad1 weights
            # (a finer-grained 64KB head was measured net-negative)
            w1_dma(S, 0, 0, 4); xg_dma(A, 0, 0, B0); xg_dma(G, 1, 0, B0)
            xg_dma(S, 2, 0, B0); w1_dma(A, 1, 0, 4); xg_dma(G, 3, 0, B0)
            w1_dma(S, 2, 0, 4); w1_dma(A, 3, 0, 4)
            G.dma_start(out=b1_sb[:], in_=b1d[:, :])
            # round 4: quad1 (m4-7)
            w1_dma(S, 0, 4, 8); w1_dma(A, 1, 4, 8); w1_dma(G, 2, 4, 8)
            # round 5-6: stage-2 weights (m8-15, 256KB per k)
            w1_dma(S, 3, 4, 8); w1_dma(A, 0, 8, 16); w1_dma(G, 1, 8, 16)
            w1_dma(S, 2, 8, 16); w1_dma(A, 3, 8, 16)
            # round 6-7: stage-3a tokens (needed ~25us in)
            xg_dma(G, 0, B0, B0 + B1)
            xg_dma(S, 1, B0, B0 + B1); xg_dma(A, 2, B0, B0 + B1)
            xg_dma(G, 3, B0, B0 + B1)
            xg_dma(S, 0, B0 + B1, cap); xg_dma(A, 1, B0 + B1, cap)
            xg_dma(G, 2, B0 + B1, cap); xg_dma(S, 3, B0 + B1, cap)
            # stage-4 weights (needed ~40us in)
            w2_dma(A, 0, 2048); w2_dma(S, 2048, 4096)
            w2_dma(G, 4096, 6144); w2_dma(A, 6144, 8192)

            # --- PE warmup: ramp the HAM clock during the DMA head ------
            ps_w = ppool.tile([128, 512], FP, tag="W", bufs=1, name="ps_warm")
            for i in range(warm):
                # alternate halves of the bank so back-to-back warms have
                # no WAW dependency -- gaps would pause the HAM ramp timer
                half = (i % 2) * 256
                nc.tensor.matmul(
                    ps_w[:, half : half + 256],
                    warm_sb[:, :128],
                    warm_sb[:, :256],
                    start=True,
                    stop=True,
                )
            if warm:
                warm_sink = cpool.tile([1, 8], FP, tag="warm_sink")
                nc.vector.tensor_copy(warm_sink[:], ps_w[0:1, 0:8])

            def relu(m, ps, lo, hi, bias=None):
                """h[m][:, lo:hi] = max(ps + bias[m], 0) in bf16 on DVE.

                DVE-only: scalar must stay a pure DMA issuer -- Tile keeps
                per-engine emission order, so relus on scalar would queue
                behind all of its DMA issues (measured: first ACT relu at
                27us, PSUM backpressure, clock down-throttle, +9us)."""
                nc.vector.tensor_scalar(
                    h_sb[m][:, lo:hi],
                    ps[:, : hi - lo],
                    (bias if bias is not None else b1_sb)[:, m : m + 1],
                    0.0,
                    mybir.AluOpType.add,
                    mybir.AluOpType.max,
                )

            # --- stage 1+2: mm1 on block0, k-major over m-quads ---------
            # Per k-pass a quad consumes one 128KB w1 chunk (+128KB xg on
            # the first quad) -- matched to the cold DMA/clock ramp.
            for q in range(4):
                ps_q = [
                    ppool.tile([128, B0], FP, tag="A", bufs=7, name=f"ps_{q}_{i}")
                    for i in range(4)
                ]
                for k in range(KD):
                    for i in range(4):
                        m = q * 4 + i
                        nc.tensor.matmul(
                            ps_q[i][:],
                            w1_sb[:, k * DFF + m * 128 : k * DFF + (m + 1) * 128],
                            xg_sb[:, k, 0:B0],
                            start=(k == 0),
                            stop=(k == KD - 1),
                        )
                    if warm and q == 0:
                        # thin deterministic padding (128-col, ~53ns): keeps
                        # the clock ramp alive if quad0 stalls on DMA, at
                        # negligible cost when it doesn't
                        nc.tensor.matmul(
                            ps_w[:, :128], warm_sb[:, :128], warm_sb[:, :128],
                            start=True, stop=True,
                        )
                for i in range(4):
                    relu(q * 4 + i, ps_q[i], 0, B0)

            # --- stage 3: mm1 on block1 + tail, m-major (w1 resident) ---
            for m in range(MD):
                ps_b = ppool.tile(
                    [128, B1], FP, tag="A", bufs=7, name=f"ps3_{m}"
                )
                for k in range(KD):
                    nc.tensor.matmul(
                        ps_b[:],
                        w1_sb[:, k * DFF + m * 128 : k * DFF + (m + 1) * 128],
                        xg_sb[:, k, B0 : B0 + B1],
                        start=(k == 0),
                        stop=(k == KD - 1),
                    )
                ps_t = ppool.tile(
                    [128, BT], FP, tag="A", bufs=7, name=f"ps3t_{m}"
                )
                for k in range(KD):
                    nc.tensor.matmul(
                        ps_t[:],
                        w1_sb[:, k * DFF + m * 128 : k * DFF + (m + 1) * 128],
                        xg_sb[:, k, B0 + B1 : cap],
                        start=(k == 0),
                        stop=(k == KD - 1),
                    )
                relu(m, ps_b, B0, B0 + B1)
                relu(m, ps_t, B0 + B1, cap)

            # --- stage 4: mm2 ------------------------------------------
            def emit_mm2(t, chunks=((0, 512),), engs=(nc.scalar,)):
                o = opool.tile([128, D_MODEL], BF, tag="o")
                for ci, (dlo, dhi) in enumerate(chunks):
                    ps2 = ppool.tile(
                        [128, dhi - dlo], FP, tag="A", bufs=7, name=f"ps2_{t}"
                    )
                    for m in range(MD):
                        nc.tensor.matmul(
                            ps2[:],
                            h_sb[m][:, t * 128 : (t + 1) * 128],
                            w2_sb[:, m * 512 + dlo : m * 512 + dhi],
                            start=(m == 0),
                            stop=(m == MD - 1),
                        )
                    nc.vector.tensor_copy(o[:, dlo:dhi], ps2[:])
                    engs[ci % len(engs)].dma_start(
                        out=out[t * 128 : (t + 1) * 128, dlo:dhi],
                        in_=o[:, dlo:dhi],
                    )

            for t in range(n_full_tiles - 1):
                emit_mm2(t)
            # transposed remainder next-to-last: its DMA (slow gpsimd
            # queue) issues while the last tile's matmuls still run
            oT = opool.tile([128, KD, rem], BF, tag="oT", bufs=1)
            for dj in range(KD):
                psT = ppool.tile(
                    [128, rem], FP, tag="A", bufs=7, name=f"psT_{dj}"
                )
                for m in range(MD):
                    nc.tensor.matmul(
                        psT[:],
                        w2_sb[:, m * 512 + dj * 128 : m * 512 + (dj + 1) * 128],
                        h_sb[m][:, full_cols:cap],
                        start=(m == 0),
                        stop=(m == MD - 1),
                    )
                nc.vector.tensor_copy(oT[:, dj, :], psT[:])
            nc.gpsimd.dma_start(out=outt[:, :, :], in_=oT[:])
            # last full tile in 4 d-chunks so each chunk's cast+DMA
            # overlaps the next chunk's matmuls; final transfers are small
            # and ride the fast HW queues
            emit_mm2(
                n_full_tiles - 1,
                chunks=((0, 192), (192, 320), (320, 448), (448, 512)),
                engs=(nc.scalar, nc.sync, nc.scalar, nc.sync),
            )
    nc.compile()
    return nc


def _route(x_flat: np.ndarray, router_w: np.ndarray):
    """Host-side replica of the reference router: top-2 + renormalized weights."""
    logits = x_flat @ router_w.T  # [L, E]
    m = logits.max(axis=-1, keepdims=True)
    p = np.exp(logits - m)
    p /= p.sum(axis=-1, keepdims=True)
    order = np.argsort(-p, axis=-1)[:, :K]  # [L, K]
    pv = np.take_along_axis(p, order, axis=-1)
    pv = pv / (pv.sum(axis=-1, keepdims=True) + 1e-9)
    return order, pv


def _stage_weights(w1_all, b1_all, w2_all, e):
    """k-major w1, m-major w2, [p, m] b1 for expert e (see build_program)."""
    w1e = np.asarray(w1_all, np.float32)[e]  # [dff, d]
    w1m = (
        w1e.reshape(MD, 128, KD, 128)  # [m, c, k, p]
        .transpose(3, 2, 0, 1)  # [p, k, m, c]
        .reshape(128, KD * DFF)
    )
    w2e = np.asarray(w2_all, np.float32)[e]  # [d, dff]
    w2m = (
        w2e.reshape(D_MODEL, MD, 128)  # [d, m, p]
        .transpose(2, 1, 0)  # [p, m, d]
        .reshape(128, MD * D_MODEL)
    )
    b1e = np.asarray(b1_all, np.float32)[e].reshape(MD, 128).T  # [p, m]
    return (
        np.ascontiguousarray(w1m).astype(NPBF),
        np.ascontiguousarray(w2m).astype(NPBF),
        np.ascontiguousarray(b1e),
    )


def _build_in_maps(x, router_w, w1_all, b1_all, w2_all, b2_all):
    """Shared staging: router + expert-parallel dispatch + per-core maps.

    Returns (cap, in_maps, idx_lists, wgt) where idx_lists[e] = token idx
    array for core e and wgt[t, e] = combine weight of expert e for token t.
    """
    x_flat = np.asarray(x, np.float32).reshape(-1, D_MODEL)
    order, pv = _route(x_flat, np.asarray(router_w, np.float32))
    idx_lists, wgt = [], np.zeros((x_flat.shape[0], E), np.float32)
    for e in range(E):
        sel = np.nonzero(order == e)
        idx_lists.append(sel[0])
        wgt[sel[0], e] = pv[sel]
    max_n = max(len(t) for t in idx_lists)
    cap = max(CAP, -(-max_n // 4) * 4)
    in_maps = []
    for e in range(E):
        toks = idx_lists[e]
        xg = np.zeros((cap, D_MODEL), np.float32)
        xg[: len(toks)] = x_flat[toks]
        w1m, w2m, b1e = _stage_weights(w1_all, b1_all, w2_all, e)
        in_maps.append(
            {
                "xgd": np.ascontiguousarray(xg.T).astype(NPBF),
                "w1d": w1m,
                "w2d": w2m,
                "b1d": b1e,
            }
        )
    return cap, in_maps, idx_lists, wgt


def _get_program(cap: int):
    if cap not in _PROG_CACHE:
        _PROG_CACHE[cap] = build_program(cap)
    return _PROG_CACHE[cap]


def kernel(x, router_w, w1_all, b1_all, w2_all, b2_all):
    from concourse.bass_utils import run_bass_kernel_spmd

    x = np.asarray(x, dtype=np.float32)
    Bb, Nn, C = x.shape

    cap, in_maps, idx_lists, wgt = _build_in_maps(
        x, router_w, w1_all, b1_all, w2_all, b2_all
    )
    nc = _get_program(cap)

    for _attempt in range(3):
        res = run_bass_kernel_spmd(nc, in_maps, core_ids=list(range(N_CORES)))
        outs = []
        bad = False
        for e in range(E):
            o = res.results[e]["out"].astype(np.float32)
            ot = res.results[e]["outt"].astype(np.float32)
            # [128, 4, rem] -> [rem, 512] with d = dj*128 + p
            o_tail = np.transpose(ot, (2, 1, 0)).reshape(ot.shape[2], -1)
            o = np.concatenate([o, o_tail], axis=0)[: len(idx_lists[e])]
            if np.isnan(o).any():
                bad = True
                break
            outs.append(o)
        if not bad:
            break
    else:
        raise RuntimeError("device output contained NaN after 3 attempts")

    b2f = np.asarray(b2_all, np.float32)
    final = np.zeros((Bb * Nn, C), np.float32)
    for e in range(E):
        toks = idx_lists[e]
        final[toks] += (outs[e] + b2f[e]) * wgt[toks, e][:, None]
    return final.reshape(Bb, Nn, C)
